# revision 84
# baseline (speedup 1.0000x reference)
"""Trainium2 Bass kernel for CustomTransformerEncoderMoELayer.

Sharding: pure data-parallel over (batch, token-half) -> 8 cores, no
collectives.  Core c handles batch c//2, tokens [512*(c%2), 512*(c%2+1)).
Each core runs an identical program on different data:

  - Q/K/V projections in feature-major layout (weights stationary),
    K/V computed for the full batch (needed for attention), Q for own tokens.
    Key/value tokens are host-permuted so the core's own tokens come first.
  - Attention: the frac factor (fj-fi)/(fi*fj+eps) equals 1/fi - 1/fj up to
    O(eps) (logit error <= 2e-4); the 1/fi term is constant per query and
    cancels in softmax, so only b_j = -(sum_b*scale)/fj remains, applied as
    a per-partition bias inside the ACT exp -- no Fs matrix, no multiplies.
    Softmax denominator obtained free via a ones-column appended to V.
  - Per-head softmax sums kept as a [16, T_own] tile (one head per
    partition): Ln/Exp run 16-lane, and the 1/sum row-broadcasts for all
    16 heads take 8 K=16 indicator matmuls instead of 16 K=1 ones.
  - LayerNorm in feature-major via ones-vector PE reductions and PE
    row-broadcasts; stats interleaved with the producing matmuls (LN1 with
    the out-projection, LN2 with the last expert) to keep the PE dense.
  - Gate in fp32 feature-major (lhsT=gate_w, one [4,512] psum, matmuls
    interleaved with LN1's affine chunks), transposed to token-major on
    the PE for the top-2 selection chain; combine weights broadcast
    through a DRAM bounce on the otherwise-idle gpsimd queue.
  - Dense MoE: all 4 experts computed for all tokens, combined with the
    (zero-masked) gate weights.  Both expert matmuls run in fp8(e4m3)
    DoubleRow perf mode (256-deep contraction per pass); weights are
    host-prescaled by 64 to stay in e4m3's normal range, with 1/64 folded
    into the ReLU activation scale (mm1) and the combine weights (mm2).
  - Output is returned feature-major [D, T_own]; the host transposes.
"""

import sys

sys.path.insert(0, "/opt/trn_rl_repo")

from contextlib import ExitStack

import ml_dtypes
import numpy as np

import concourse.bass as bass
import concourse.tile as tile
from concourse import bacc, mybir
from concourse.bass_utils import run_bass_kernel_spmd
from concourse.masks import make_identity

AF = mybir.ActivationFunctionType
ALU = mybir.AluOpType
F32 = mybir.dt.float32
BF16 = mybir.dt.bfloat16
F8 = mybir.dt.float8e4
DR = mybir.MatmulPerfMode.DoubleRow
# SwInterleave: same math as DoubleRow but weights pre-interleaved per column
# pair and column-reversed (the hardware's native LDW streaming order)
USE_SWI = True
DRW = mybir.MatmulPerfMode.DoubleRowSwInterleave if USE_SWI else DR
BF16_NP = ml_dtypes.bfloat16
FP8_NP = ml_dtypes.float8_e4m3

B, T, D = 4, 1024, 1024
H, HD, FF, E = 16, 64, 4096, 4
P = 128
TOK = 512  # tokens per core
NDC = D // P  # 8 feature chunks
NJC = T // P  # 8 key-token chunks
NFC = FF // P  # 32 FF chunks
NOC = D // P  # 8 output feature chunks
NTC = TOK // P  # 4 own-token chunks
N_CORES = 8
EPS_ATTN, EPS_LN = 1e-8, 1e-5
S_W = 64.0  # fp8 weight prescale (moves w into e4m3 normal range)
S_QK = 512.0  # prescale for wq (already carries the hd^-0.5 attention scale)


def _declare_io(nc):
    d = {}

    def din(name, shape, dtype):
        d[name] = nc.dram_tensor(name, shape, dtype, kind="ExternalInput").ap()

    din("srcT_full", [D, T], BF16)
    din("srcT8_full", [D, T], F8)
    din("res_own", [D, TOK], F32)
    din("wq", [D, D], F8)
    din("wk", [D, D], F8)
    din("wv", [D, D], BF16)
    din("wo", [NOC, P, D], BF16)
    # [D]-vector params arrive host-prearranged [P, NDC] (one contiguous
    # run per partition: 128 DMA descriptors instead of 1024 4-byte ones)
    din("bq", [P, NDC], F32)
    din("bk", [P, NDC], F32)
    din("bv", [D], F32)
    din("bo", [P, NDC], F32)
    din("gate_wbf", [P, NDC, E], BF16)
    din("gate_wr", [P, NDC, E], BF16)
    din("gate_b", [E], F32)
    din("ew1", [E, NFC, P, D], F8)
    din("eb1", [P, E, NFC], F32)
    din("ew2", [E, NOC, P, FF], F8)
    din("eb2", [P, E, NOC], F32)
    din("ln1g", [P, NDC], F32)
    din("ln1b", [P, NDC], F32)
    din("ln2g", [P, NDC], F32)
    din("ln2b", [P, NDC], F32)
    din("bj", [P, NJC], F32)
    d["out"] = nc.dram_tensor("out", [D, TOK], BF16, kind="ExternalOutput").ap()
    return d


def _bcast_ap(base, parts, free_len):
    """AP reading `free_len` contiguous elements at base, replicated on
    `parts` partitions (partition step 0)."""
    return bass.AP(tensor=base.tensor, offset=base.offset, ap=[[0, parts], [1, free_len]])


def _fm_layernorm(tc, nc, x_in, g_sb, b_sb, out_f32, out_q8, cst,
                  sq_pool, row_sb, bc_sb, producer=None, after_affine=None):
    """LayerNorm over the feature (partition x chunk) axis, feature-major.

    x_in(dc) -> [P, TOK] f32 view of chunk dc.  producer(dc), if given, emits
    the instructions that produce x_in(dc) (stats matmuls interleave with it).
    Stats run on bf16 casts (PE ones-reduction at full rate; the averaging
    washes out the rounding).  after_affine(dc) runs after each output chunk.
    """
    with tc.tile_pool(name="ln_row_ps", bufs=2, space="PSUM") as row_ps, \
         tc.tile_pool(name="ln_bc_ps", bufs=1, space="PSUM") as bc_ps:
        sum_ps = row_ps.tile([1, TOK], F32, name="lnrow", tag="lnrow")
        sumsq_ps = row_ps.tile([1, TOK], F32, name="lnrow", tag="lnrow")
        for dc in range(NDC):
            if producer is not None:
                producer(dc)
            xb = sq_pool.tile([P, TOK], BF16, name="xb", tag="xb")
            nc.vector.tensor_copy(xb, x_in(dc))
            nc.tensor.matmul(sum_ps, lhsT=cst["ones_col_bf"], rhs=xb,
                             start=(dc == 0), stop=(dc == NDC - 1))
            sqb = sq_pool.tile([P, TOK], BF16, name="sqb", tag="sqb")
            nc.vector.tensor_mul(sqb, xb, xb)
            nc.tensor.matmul(sumsq_ps, lhsT=cst["ones_col_bf"], rhs=sqb,
                             start=(dc == 0), stop=(dc == NDC - 1))
        stats2 = row_sb.tile([1, 2, TOK], F32, name="stats2", tag="stats2")
        mu_row = stats2[:, 0, :]
        nc.scalar.mul(mu_row, sum_ps, 1.0 / D)
        st_bc_ps = bc_ps.tile([P, 2, TOK], F32, name="lnbc", tag="lnbc")
        st_bc = bc_sb.tile([P, 2, TOK], F32, name="st_bc", tag="st_bc")
        mu_bc = st_bc[:, 0, :]
        rstd_bc = st_bc[:, 1, :]
        # mu broadcast + copy launch immediately; the variance/rstd chain
        # overlaps them and the early (x - mu) subs below
        nc.tensor.matmul(st_bc_ps[:, 0, :], lhsT=cst["ones_row"], rhs=stats2[:, 0, :],
                         start=True, stop=True)
        nc.scalar.copy(mu_bc, st_bc_ps[:, 0, :])
        musq = row_sb.tile([1, TOK], F32, name="musq", tag="musq")
        nc.vector.tensor_mul(musq, mu_row, mu_row)
        var_row = row_sb.tile([1, TOK], F32, name="var_row", tag="var_row")
        nc.vector.scalar_tensor_tensor(out=var_row, in0=sumsq_ps, scalar=1.0 / D,
                                       in1=musq, op0=ALU.mult, op1=ALU.subtract)
        lnv_row = row_sb.tile([1, TOK], F32, name="lnv_row", tag="lnv_row")
        nc.scalar.activation(lnv_row, var_row, AF.Ln, bias=cst["eps_row"])
        # rstd = (var+eps)^-0.5 via exp/ln: stays in the natural_log_exp ACT
        # table set (no table switch around the attention/gate exps) and
        # avoids the low-precision Sqrt table
        nc.scalar.activation(stats2[:, 1, :], lnv_row, AF.Exp, scale=-0.5)
        nc.tensor.matmul(st_bc_ps[:, 1, :], lhsT=cst["ones_row"], rhs=stats2[:, 1, :],
                         start=True, stop=True)
        nc.scalar.copy(rstd_bc, st_bc_ps[:, 1, :])

        for dc in range(NDC):
            t1 = sq_pool.tile([P, TOK], F32, name="sq", tag="sq")
            nc.vector.tensor_sub(t1, x_in(dc), mu_bc)
            t2 = sq_pool.tile([P, TOK], F32, name="sq", tag="sq")
            nc.vector.tensor_mul(t2, t1, rstd_bc)
            nc.scalar.activation(out_f32(dc), t2, AF.Identity,
                                 bias=b_sb[:, dc:dc + 1], scale=g_sb[:, dc:dc + 1])
            if out_q8 is not None:
                nc.vector.tensor_copy(out_q8(dc), out_f32(dc))
            if after_affine is not None:
                after_affine(dc)


def _emit_kernel(tc, nc, io):
    stk = ExitStack()
    with stk:
        # ---------------- constants / params (live whole kernel) ----------
        cpool = stk.enter_context(tc.tile_pool(name="const", bufs=1))
        cst = {}
        cst["ones_col_bf"] = cpool.tile([P, 1], BF16, name="ones_col_bf", tag="ones_col_bf")
        nc.vector.memset(cst["ones_col_bf"], 1.0)
        cst["ones_row"] = cpool.tile([1, P], F32, name="ones_row", tag="ones_row")
        nc.vector.memset(cst["ones_row"], 1.0)
        ident = cpool.tile([P, P], F32, name="ident", tag="ident")
        make_identity(nc, ident)
        cst["eps_row"] = cpool.tile([1, 1], F32, name="eps_row", tag="eps_row")
        nc.vector.memset(cst["eps_row"], EPS_LN)

        def col_tile(name, cols=NDC):
            return cpool.tile([P, cols], F32, name=name, tag=name)

        bq_sb = col_tile("bq")
        bk_sb = col_tile("bk")
        bo_sb = col_tile("bo")
        ln1g_sb = col_tile("ln1g")
        ln1b_sb = col_tile("ln1b")
        ln2g_sb = col_tile("ln2g")
        ln2b_sb = col_tile("ln2b")
        bj_sb = cpool.tile([P, NJC], F32, name="bj", tag="bj")
        eb1_sb = cpool.tile([P, E, NFC], F32, name="eb1", tag="eb1")
        eb2_sb = cpool.tile([P, E, NOC], F32, name="eb2", tag="eb2")
        gate_wbf_sb = cpool.tile([P, NDC, E], BF16, name="gate_wbf", tag="gate_wbf")
        gate_wr_sb = cpool.tile([P, NDC, E], BF16, name="gate_wr", tag="gate_wr")
        gate_b_bc = cpool.tile([P, E], F32, name="gate_b", tag="gate_b")
        bv_bc = cpool.tile([P, D], BF16, name="bv_bc", tag="bv_bc")

        def emit_const_loads():
            # all consts arrive host-prearranged partition-major: each DMA is
            # 128 contiguous runs, so the whole block is ~1K descriptors
            for t, name in ((bq_sb, "bq"), (bk_sb, "bk"), (bo_sb, "bo"),
                            (ln1g_sb, "ln1g"), (ln1b_sb, "ln1b"),
                            (ln2g_sb, "ln2g"), (ln2b_sb, "ln2b"),
                            (bj_sb, "bj"), (eb1_sb, "eb1"), (eb2_sb, "eb2"),
                            (gate_wbf_sb, "gate_wbf"), (gate_wr_sb, "gate_wr")):
                nc.sync.dma_start(out=t, in_=io[name])
            nc.sync.dma_start(out=gate_b_bc, in_=_bcast_ap(io["gate_b"], P, E))
            nc.gpsimd.dma_start(out=bv_bc, in_=_bcast_ap(io["bv"], P, D))

        # ---------------- persistent activations --------------------------
        per = stk.enter_context(tc.tile_pool(name="persist", bufs=1))
        xres = per.tile([P, NDC, TOK], F32, name="xres", tag="xres")
        xln = per.tile([P, NDC, TOK], F32, name="xln", tag="xln")
        xq8 = per.tile([P, NDC, TOK], F8, name="xq8", tag="xq8")
        ff = per.tile([P, NOC, TOK], F32, name="ff", tag="ff")

        sq_pool = stk.enter_context(tc.tile_pool(name="sq", bufs=3))
        row_sb = stk.enter_context(tc.tile_pool(name="row_sb", bufs=1))
        bc_sb = stk.enter_context(tc.tile_pool(name="bc_sb", bufs=1))
        # ================== attention ======================================
        with ExitStack() as astk:
            apool = astk.enter_context(tc.tile_pool(name="attn_sb", bufs=1))
            # Q zero-padded per head: even heads in rows 0:64 (zeros above),
            # odd heads in rows 64:128 (zeros below).  QK then contracts over
            # all 128 rows with the pair's shared K tile: the zeros kill the
            # other head's contribution, and the full-K matmul keeps the PE
            # activity monitor warm (K=64 streams throttle to half clock).
            QTp = apool.tile([P, H, TOK], BF16, name="QTp", tag="QTp")
            KT = apool.tile([P, NDC, T], BF16, name="KT", tag="KT")
            Vp = apool.tile([P, NJC, H, HD + 1], BF16, name="Vp", tag="Vp")
            attnT = apool.tile([P, NDC, TOK], BF16, name="attnT", tag="attnT")
            for jc in range(NJC):
                nc.vector.memset(Vp[:, jc, :, HD:HD + 1], 1.0)

            # ---- projections ----
            with ExitStack() as pstk:
                ppool = pstk.enter_context(tc.tile_pool(name="proj_sb", bufs=1))
                wpool = pstk.enter_context(tc.tile_pool(name="w_sb", bufs=2))
                mm_ps = pstk.enter_context(tc.tile_pool(name="proj_mm", bufs=3, space="PSUM"))
                srcT = ppool.tile([P, NDC, T], BF16, name="srcT", tag="srcT")
                srcT8 = ppool.tile([P, NDC, T], F8, name="srcT8", tag="srcT8")
                src_rearr = io["srcT_full"].rearrange("(c p) t -> p c t", p=P)
                src8_rearr = io["srcT8_full"].rearrange("(c p) t -> p c t", p=P)

                def load_w(name, dtype):
                    w = wpool.tile([P, NDC, D], dtype, tag="w", name="w")
                    w_rearr = io[name].rearrange("(c p) o -> p c o", p=P)
                    for dc in range(NDC):
                        nc.sync.dma_start(out=w[:, dc:dc + 1, :], in_=w_rearr[:, dc:dc + 1, :])
                    return w

                # one engine queue per input stream so K (wk8+srcT8), Q (wq8)
                # and V (srcT bf16 + wv) are never serialized behind each
                # other: sync carries the weights, scalar srcT8, gpsimd srcT
                wk = wpool.tile([P, NDC, D], F8, tag="w", name="w")
                wk_rearr = io["wk"].rearrange("(c p) o -> p c o", p=P)
                for dc in range(NDC):
                    # K inputs split into partition halves (twice the rings in
                    # flight, halving per-chunk descriptor serialization)
                    for pr in (slice(0, 64), slice(64, P)):
                        nc.sync.dma_start(out=wk[pr, dc:dc + 1, :], in_=wk_rearr[pr, dc:dc + 1, :])
                        nc.scalar.dma_start(out=srcT8[pr, dc:dc + 1, :], in_=src8_rearr[pr, dc:dc + 1, :])
                    nc.gpsimd.dma_start(out=srcT[:, dc:dc + 1, :], in_=src_rearr[:, dc:dc + 1, :])
                emit_const_loads()

                # K projection: feature-major, full batch, fp8 DoubleRow
                # (wk host-prescaled by S_W; descale rides the ACT bias pass)
                for oc in range(NDC):
                    for th in range(T // TOK):
                        ps = mm_ps.tile([P, TOK], F32, name="mm", tag="mm")
                        for d2 in range(NDC // 2):
                            nc.tensor.matmul(ps, lhsT=wk[:, 2 * d2:2 * d2 + 2, oc * P:(oc + 1) * P],
                                             rhs=srcT8[:, 2 * d2:2 * d2 + 2, th * TOK:(th + 1) * TOK],
                                             start=(d2 == 0), stop=(d2 == NDC // 2 - 1),
                                             perf_mode=DR)
                        nc.scalar.activation(KT[:, oc, th * TOK:(th + 1) * TOK], ps,
                                             AF.Identity, bias=bk_sb[:, oc:oc + 1],
                                             scale=1.0 / S_W)
                # Q projection (own tokens = first TOK of the permuted order;
                # wq/bq pre-scaled by hd^-0.5 on host, wq by S_QK for fp8)
                for h in range(H):
                    z0 = (h % 2) * HD  # zeros live in the OTHER half
                    nc.vector.memset(QTp[HD - z0:P - z0, h, :], 0.0)
                wq = load_w("wq", F8)
                for oc in range(NDC):
                    ps = mm_ps.tile([P, TOK], F32, name="mm", tag="mm")
                    for d2 in range(NDC // 2):
                        nc.tensor.matmul(ps, lhsT=wq[:, 2 * d2:2 * d2 + 2, oc * P:(oc + 1) * P],
                                         rhs=srcT8[:, 2 * d2:2 * d2 + 2, 0:TOK],
                                         start=(d2 == 0), stop=(d2 == NDC // 2 - 1),
                                         perf_mode=DR)
                    nc.scalar.activation(QTp[0:HD, 2 * oc, :], ps[0:HD, :],
                                         AF.Identity, bias=bq_sb[0:HD, oc:oc + 1],
                                         scale=1.0 / S_QK)
                    nc.scalar.activation(QTp[HD:P, 2 * oc + 1, :], ps[HD:P, :],
                                         AF.Identity, bias=bq_sb[HD:P, oc:oc + 1],
                                         scale=1.0 / S_QK)
                # V projection: token-major (src chunk stationary), full batch
                wv = load_w("wv", BF16)
                for jc in range(NJC):
                    for nh in range(D // TOK):
                        ps = mm_ps.tile([P, TOK], F32, name="mm", tag="mm")
                        for dc in range(NDC):
                            nc.tensor.matmul(ps, lhsT=srcT[:, dc, jc * P:(jc + 1) * P],
                                             rhs=wv[:, dc, nh * TOK:(nh + 1) * TOK],
                                             start=(dc == 0), stop=(dc == NDC - 1))
                        nc.vector.tensor_add(
                            Vp[:, jc, nh * 8:(nh + 1) * 8, 0:HD],
                            ps.rearrange("p (a b) -> p a b", a=8),
                            bv_bc[:, nh * TOK:(nh + 1) * TOK].rearrange("p (a b) -> p a b", a=8))

            # prefetch the residual while QK/PV runs (persist tile: no extra SBUF)
            res_rearr = io["res_own"].rearrange("(c p) t -> p c t", p=P)
            for dc in range(NDC):
                nc.sync.dma_start(out=xres[:, dc:dc + 1, :], in_=res_rearr[:, dc:dc + 1, :])



            # ---- attention core: head pairs packed via tile_position ----
            # probs ∝ exp(K^T Q * scale + b_j): the frac factor's query term
            # cancels in softmax and the key term b_j rides the ACT exp as a
            # per-partition bias -- no Fs matrix multiply at all.
            sums_pool = astk.enter_context(tc.tile_pool(name="sums_sb", bufs=1))
            sums_all = sums_pool.tile([1, H, TOK], F32, name="sums_all", tag="sums_all")
            with ExitStack() as astk2:
                e0_pool = astk2.enter_context(tc.tile_pool(name="e0_sb", bufs=6))
                s_ps_pool = astk2.enter_context(tc.tile_pool(name="s_ps", bufs=2, space="PSUM"))
                att_ps_pool = astk2.enter_context(tc.tile_pool(name="att_ps", bufs=3, space="PSUM"))
                bc_ps_pool = astk2.enter_context(tc.tile_pool(name="bc_ps", bufs=1, space="PSUM"))
                rcp_pool = astk2.enter_context(tc.tile_pool(name="rcp_sb", bufs=2))

                def norm_chunk(dch):
                    # pair dch's heads fill attnT chunk dch: broadcast the two
                    # raw sums, one 128-lane reciprocal + multiply -- rides
                    # inside the ring so out-proj starts right after pair 7
                    ha, hb = 2 * dch, 2 * dch + 1
                    bc_ps = bc_ps_pool.tile([P, TOK], F32, name="bc", tag="bc")
                    nc.tensor.matmul(bc_ps[0:HD, :], lhsT=cst["ones_row"][:, 0:HD],
                                     rhs=sums_all[0:1, ha, :], start=True, stop=True)
                    nc.tensor.matmul(bc_ps[HD:P, :], lhsT=cst["ones_row"][:, 0:HD],
                                     rhs=sums_all[0:1, hb, :], start=True, stop=True)
                    rcp = rcp_pool.tile([P, TOK], F32, name="rcp", tag="rcp")
                    nc.vector.reciprocal(rcp, bc_ps)
                    nc.vector.tensor_tensor(attnT[:, dch, :], attnT[:, dch, :], rcp,
                                            op=ALU.mult)

                for hp2 in range(H // 2):
                    ha, hb = 2 * hp2, 2 * hp2 + 1
                    att_a = att_ps_pool.tile([HD + 1, TOK], F32, name="att", tag="att")
                    att_b = att_ps_pool.tile([HD + 1, TOK], F32, name="att", tag="att")
                    exp_tiles = []

                    def emit_pv(jc, att_a=att_a, att_b=att_b, exp_tiles=exp_tiles,
                                ha=ha, hb=hb):
                        et = exp_tiles[jc]
                        nc.tensor.matmul(att_a, lhsT=Vp[:, jc, ha, :], rhs=et[:, 0, :],
                                         start=(jc == 0), stop=(jc == NJC - 1))
                        nc.tensor.matmul(att_b, lhsT=Vp[:, jc, hb, :], rhs=et[:, 1, :],
                                         start=(jc == 0), stop=(jc == NJC - 1))

                    for jc in range(NJC):
                        s_ps = s_ps_pool.tile([P, 2, TOK], F32, name="s", tag="s")
                        nc.tensor.matmul(s_ps[:, 0, :], lhsT=KT[:, hp2, jc * P:(jc + 1) * P],
                                         rhs=QTp[:, ha, :], start=True, stop=True)
                        nc.tensor.matmul(s_ps[:, 1, :], lhsT=KT[:, hp2, jc * P:(jc + 1) * P],
                                         rhs=QTp[:, hb, :], start=True, stop=True)
                        et = e0_pool.tile([P, 2, TOK], BF16, name="e0", tag="e0")
                        nc.scalar.activation(et, s_ps, AF.Exp, bias=bj_sb[:, jc:jc + 1])
                        exp_tiles.append(et)
                        if jc >= 2:
                            emit_pv(jc - 2)
                    emit_pv(NJC - 2)
                    emit_pv(NJC - 1)
                    # stage unnormalized head outputs + softmax sums; all
                    # normalization is batched after the loop (one Ln + one
                    # Exp for all 16 heads -- per-pair Ln/Exp thrashed the
                    # ACT table sets, ~3 TABLE_LOADs per pair)
                    for i, (att, h) in enumerate(((att_a, ha), (att_b, hb))):
                        nc.vector.tensor_copy(attnT[i * HD:(i + 1) * HD, hp2, :], att[0:HD, :])
                        nc.vector.tensor_copy(sums_all[0:1, h, :], att[HD:HD + 1, :])
                    norm_chunk(hp2)

            # gate psum lives from LN1's affine chunks to the gate section
            gate_ps_pool = stk.enter_context(tc.tile_pool(name="gate_ps", bufs=1, space="PSUM"))
            g_ps = gate_ps_pool.tile([E, TOK], F32, name="g_ps", tag="g_ps")

            # ---- output projection + residual + LN1 (stats interleaved) ----
            with ExitStack() as ostk:
                mm_ps = ostk.enter_context(tc.tile_pool(name="out_mm", bufs=2, space="PSUM"))
                wo_pool = ostk.enter_context(tc.tile_pool(name="wo_sb", bufs=3))
                # wo streams as per-oc column tiles: the first psum group
                # needs 256KB, not the whole 2MB tensor
                wo_tiles = []
                for oc in range(NOC):
                    woc = wo_pool.tile([P, NDC, P], BF16, name="woc", tag="woc")
                    nc.sync.dma_start(out=woc, in_=io["wo"][oc].rearrange("p (c n) -> p c n", c=NDC))
                    wo_tiles.append(woc)

                def ln1_producer(oc):
                    ps = mm_ps.tile([P, TOK], F32, name="mm", tag="mm")
                    for dc in range(NDC):
                        nc.tensor.matmul(ps, lhsT=wo_tiles[oc][:, dc, :],
                                         rhs=attnT[:, dc, :],
                                         start=(dc == 0), stop=(dc == NDC - 1))
                    nc.vector.scalar_tensor_tensor(out=xres[:, oc, :], in0=ps,
                                                   scalar=bo_sb[:, oc:oc + 1],
                                                   in1=xres[:, oc, :],
                                                   op0=ALU.add, op1=ALU.add)

                _fm_layernorm(tc, nc, lambda dc: xres[:, dc, :], ln1g_sb, ln1b_sb,
                              lambda dc: xln[:, dc, :], lambda dc: xq8[:, dc, :],
                              cst, sq_pool, row_sb, bc_sb, producer=ln1_producer)

        # combine weights (row-broadcast), needed from gate through MoE
        cbc_pool = stk.enter_context(tc.tile_pool(name="cbc_pool", bufs=1))
        cbc = cbc_pool.tile([P, E, TOK], F32, name="cbc", tag="cbc")

        # bf16 value + residual of xln for the gate's split-bf16 logits
        # (xbf@gbf + xbf@gr + xr@gbf; products are exact in the f32 psum, so
        # only the ~2^-18 xr@gr term is dropped)
        xsplit = stk.enter_context(tc.tile_pool(name="xsplit", bufs=1))
        xbf = xsplit.tile([P, NDC, TOK], BF16, name="xbf", tag="xbf")
        xr = xsplit.tile([P, NDC, TOK], BF16, name="xr", tag="xr")
        for dc in range(NDC):
            nc.vector.tensor_copy(xbf[:, dc, :], xln[:, dc, :])
            nc.vector.tensor_sub(xr[:, dc, :], xln[:, dc, :], xbf[:, dc, :])

        # ================== MoE (dense, all experts) + LN2 + output ========
        with ExitStack() as mstk:
            h_pool = mstk.enter_context(tc.tile_pool(name="hT", bufs=2))
            w1_pool = mstk.enter_context(tc.tile_pool(name="ew1_sb", bufs=10))
            w2_pool = mstk.enter_context(tc.tile_pool(name="ew2_sb", bufs=3))
            ytmp_pool = mstk.enter_context(tc.tile_pool(name="ytmp", bufs=2))
            mm_ps = mstk.enter_context(tc.tile_pool(name="moe_mm", bufs=2, space="PSUM"))

            def expert_w1_load(e):
                tiles = []
                for fc in range(NFC):
                    w1 = w1_pool.tile([P, NDC, P], F8, name="w1", tag="w1")
                    nc.sync.dma_start(out=w1, in_=io["ew1"][e, fc].rearrange("p (c n) -> p c n", c=NDC))
                    tiles.append(w1)
                return tiles

            def expert_h(e, w1_tiles, extra=None):
                h_all = h_pool.tile([P, NFC, TOK], F8, name="h_all", tag="h_all")
                for fc in range(NFC):
                    w1 = w1_tiles[fc]
                    h_ps = mm_ps.tile([P, TOK], F32, name="mm", tag="mm")
                    for d2 in range(NDC // 2):
                        nc.tensor.matmul(h_ps, lhsT=w1[:, 2 * d2:2 * d2 + 2, :],
                                         rhs=xq8[:, 2 * d2:2 * d2 + 2, :],
                                         start=(d2 == 0), stop=(d2 == NDC // 2 - 1),
                                         perf_mode=DRW)
                    nc.scalar.activation(h_all[:, fc, :], h_ps, AF.Relu,
                                         bias=eb1_sb[:, e, fc:fc + 1], scale=1.0 / S_W)
                    if extra is not None:
                        extra(fc)
                return h_all

            def w2_load(e, oc):
                w2 = w2_pool.tile([P, NFC, P], F8, name="w2", tag="w2")
                nc.gpsimd.dma_start(out=w2, in_=io["ew2"][e, oc].rearrange("p (c n) -> p c n", c=NFC))
                return w2

            def expert_y(e, h_all, oc, w2=None):
                if w2 is None:
                    w2 = w2_load(e, oc)
                y_ps = mm_ps.tile([P, TOK], F32, name="mm", tag="mm")
                for f2 in range(NFC // 2):
                    nc.tensor.matmul(y_ps, lhsT=w2[:, 2 * f2:2 * f2 + 2, :],
                                     rhs=h_all[:, 2 * f2:2 * f2 + 2, :],
                                     start=(f2 == 0), stop=(f2 == NFC // 2 - 1),
                                     perf_mode=DRW)
                if e == 0:
                    nc.vector.scalar_tensor_tensor(out=ff[:, oc, :], in0=y_ps,
                                                   scalar=eb2_sb[:, e, oc:oc + 1],
                                                   in1=cbc[:, e, :], op0=ALU.add, op1=ALU.mult)
                else:
                    yt = ytmp_pool.tile([P, TOK], F32, name="yt", tag="yt")
                    nc.vector.scalar_tensor_tensor(out=yt, in0=y_ps,
                                                   scalar=eb2_sb[:, e, oc:oc + 1],
                                                   in1=cbc[:, e, :], op0=ALU.add, op1=ALU.mult)
                    nc.vector.tensor_add(ff[:, oc, :], ff[:, oc, :], yt)

            # prefetch expert-0 weights before the gate's DVE chain queues up
            w1_first = expert_w1_load(0)

            def gate_mm(fc):
                # split-bf16 gate logit matmuls ride inside expert-0's h stream
                if fc < NDC:
                    for i, (w, x) in enumerate(((gate_wbf_sb, xbf), (gate_wr_sb, xbf),
                                                (gate_wbf_sb, xr))):
                        nc.tensor.matmul(g_ps, lhsT=w[:, fc, :], rhs=x[:, fc, :],
                                         start=(fc == 0 and i == 0),
                                         stop=(fc == NDC - 1 and i == 2))

            h_all0 = expert_h(0, w1_first, extra=gate_mm)
            # expert-0's first two w2 tiles stream before the gate's gpsimd
            # DMAs (only 2 outstanding so the cbc bounce is never queued
            # behind a slot-blocked w2 load)
            w2_pre = [w2_load(0, 0), w2_load(0, 1)]

            # ================== gate + top-2 routing (fp32) ====================
            # logits accumulated during expert-0 h; transpose to token-major
            # on the PE for the selection chain.  DRAM bounce rides the
            # gpsimd queue so it is not stuck behind w1/w2 streams.
            with ExitStack() as gstk:
                gsb = gstk.enter_context(tc.tile_pool(name="gate_sb", bufs=3))
                gsmall = gstk.enter_context(tc.tile_pool(name="gate_small", bufs=2))
                tp4_ps_pool = gstk.enter_context(tc.tile_pool(name="tp4_ps", bufs=2, space="PSUM"))
                dram_pool = gstk.enter_context(tc.tile_pool(name="cdram", bufs=1, space="DRAM"))
                c_dram = dram_pool.tile([E, TOK], F32, name="c_dram", tag="c_dram")

                lgT = gsb.tile([E, TOK], F32, name="lgT", tag="lgT")
                nc.scalar.copy(lgT, g_ps)
                for tcn in range(NTC):
                    tp4 = tp4_ps_pool.tile([P, E], F32, name="tp4", tag="tp4")
                    nc.tensor.transpose(tp4, lgT[:, tcn * P:(tcn + 1) * P], ident[0:E, 0:E])
                    lg = gsb.tile([P, E], F32, name="lg", tag="lg")
                    nc.vector.tensor_add(lg, tp4, gate_b_bc)
                    m = gsmall.tile([P, 1], F32, name="m", tag="m")
                    nc.vector.reduce_max(m, lg, axis=mybir.AxisListType.X)
                    negm = gsmall.tile([P, 1], F32, name="negm", tag="negm")
                    nc.vector.tensor_scalar(negm, m, -1.0, None, op0=ALU.mult)
                    et = gsb.tile([P, E], F32, name="et", tag="et")
                    nc.scalar.activation(et, lg, AF.Exp, bias=negm)
                    ssum = gsmall.tile([P, 1], F32, name="ssum", tag="ssum")
                    nc.vector.reduce_sum(ssum, et, axis=mybir.AxisListType.X)
                    rinv = gsmall.tile([P, 1], F32, name="rinv", tag="rinv")
                    nc.vector.reciprocal(rinv, ssum)
                    pt = gsb.tile([P, E], F32, name="pt", tag="pt")
                    nc.vector.tensor_scalar(pt, et, rinv, None, op0=ALU.mult)
                    # pairwise is_ge: [ge01, ge12, ge23], [ge02, ge13], [ge03]
                    ge1 = gsb.tile([P, 3], F32, name="ge1", tag="ge1")
                    nc.vector.tensor_tensor(ge1, pt[:, 0:3], pt[:, 1:4], op=ALU.is_ge)
                    ge2 = gsb.tile([P, 2], F32, name="ge2", tag="ge2")
                    nc.vector.tensor_tensor(ge2, pt[:, 0:2], pt[:, 2:4], op=ALU.is_ge)
                    ge3 = gsb.tile([P, 1], F32, name="ge3", tag="ge3")
                    nc.vector.tensor_tensor(ge3, pt[:, 0:1], pt[:, 3:4], op=ALU.is_ge)
                    cnt = gsb.tile([P, E], F32, name="cnt", tag="cnt")
                    tmp = gsmall.tile([P, 1], F32, name="tmp", tag="tmp")
                    # cnt0 = 3 - ge01 - ge02 - ge03
                    nc.vector.tensor_add(tmp, ge1[:, 0:1], ge2[:, 0:1])
                    nc.vector.tensor_add(tmp, tmp, ge3[:, 0:1])
                    nc.vector.tensor_scalar(cnt[:, 0:1], tmp, -1.0, 3.0, op0=ALU.mult, op1=ALU.add)
                    # cnt1 = 2 + ge01 - ge12 - ge13
                    nc.vector.tensor_sub(tmp, ge1[:, 0:1], ge1[:, 1:2])
                    nc.vector.tensor_sub(tmp, tmp, ge2[:, 1:2])
                    nc.vector.tensor_scalar(cnt[:, 1:2], tmp, 2.0, None, op0=ALU.add)
                    # cnt2 = 1 + ge02 + ge12 - ge23
                    nc.vector.tensor_add(tmp, ge2[:, 0:1], ge1[:, 1:2])
                    nc.vector.tensor_sub(tmp, tmp, ge1[:, 2:3])
                    nc.vector.tensor_scalar(cnt[:, 2:3], tmp, 1.0, None, op0=ALU.add)
                    # cnt3 = ge03 + ge13 + ge23
                    nc.vector.tensor_add(tmp, ge3[:, 0:1], ge2[:, 1:2])
                    nc.vector.tensor_add(cnt[:, 3:4], tmp, ge1[:, 2:3])
                    mask = gsb.tile([P, E], F32, name="mask", tag="mask")
                    nc.vector.tensor_scalar(mask, cnt, 1.5, None, op0=ALU.is_le)
                    csb = gsb.tile([P, E], F32, name="csb", tag="csb")
                    # 1/S_W folded here: cbc carries combine/S_W so the
                    # host-prescaled fp8 mm2 psum needs no extra descale op
                    nc.vector.scalar_tensor_tensor(out=csb, in0=pt, scalar=1.0 / S_W,
                                                   in1=mask, op0=ALU.mult, op1=ALU.mult)
                    nc.gpsimd.dma_start(out=c_dram[:, tcn * P:(tcn + 1) * P].rearrange("e t -> t e"),
                                        in_=csb)
                for e in range(E):
                    nc.gpsimd.dma_start(out=cbc[:, e, :], in_=_bcast_ap(c_dram[e:e + 1, :], P, TOK))

            # experts pipeline: w1 for expert e+1 streams while expert e's
            # second matmul runs (w2 rides the vector queue, w1 the sync one)
            w1_next = expert_w1_load(1)
            for e in range(E - 1):
                h_all = h_all0 if e == 0 else expert_h(e, w1_next)
                if e > 0:
                    w1_next = expert_w1_load(e + 1)
                for oc in range(NOC):
                    expert_y(e, h_all, oc,
                             w2=w2_pre[oc] if (e == 0 and oc < len(w2_pre)) else None)
            # last expert: y-chunks + x2 = xln + ff feed LN2 stats directly
            h_last = expert_h(E - 1, w1_next)

            def ln2_producer(oc):
                expert_y(E - 1, h_last, oc)
                nc.vector.tensor_add(ff[:, oc, :], ff[:, oc, :], xln[:, oc, :])

            obf = h_pool.tile([P, NDC, TOK], BF16, name="obf", tag="obf", bufs=1)
            out_rearr = io["out"].rearrange("(c p) t -> p c t", p=P)

            def ln2_after(dc):
                # output leaves feature-major bf16 (halves on two queues so
                # the drain overlaps the remaining affine chunks)
                nc.sync.dma_start(out=out_rearr[0:64, dc:dc + 1, :], in_=obf[0:64, dc:dc + 1, :])
                nc.scalar.dma_start(out=out_rearr[64:P, dc:dc + 1, :], in_=obf[64:P, dc:dc + 1, :])

            _fm_layernorm(tc, nc, lambda dc: ff[:, dc, :], ln2g_sb, ln2b_sb,
                          lambda dc: obf[:, dc, :], None,
                          cst, sq_pool, row_sb, bc_sb,
                          producer=ln2_producer, after_affine=ln2_after)


_CACHE = {}


def _build():
    if "nc" in _CACHE:
        return _CACHE["nc"]
    nc = bacc.Bacc("TRN2", target_bir_lowering=False, debug=False, num_devices=N_CORES)
    io = _declare_io(nc)
    with tile.TileContext(nc) as tc:
        _emit_kernel(tc, nc, io)
    nc.compile()
    _CACHE["nc"] = nc
    return nc


def _q8(w):
    return np.clip(np.asarray(w, np.float32) * S_W, -240.0, 240.0).astype(FP8_NP)


def _swi(a, npair):
    """SwInterleave weight layout: per k-pair, columns interleaved
    (A,B alternating) and column-reversed -- flat[d2, 2i+j] = chunk_{2d2+j}
    column (127-i).  No-op passthrough shape-wise."""
    if not USE_SWI:
        return a
    shp = a.shape[:-1]
    v = a.reshape(*shp, npair, 2, P)[..., ::-1]
    v = np.swapaxes(v, -1, -2)
    return np.ascontiguousarray(v.reshape(*shp, npair * 2 * P))


def prep_in_maps(inputs):
    f32 = np.float32
    src = np.asarray(inputs["src"], f32)
    frac = np.asarray(inputs["frac"], f32)
    attn_bias = np.asarray(inputs["attn_bias"], f32)
    scale = f32(HD ** -0.5)
    sum_b = np.sum(attn_bias, dtype=f32)

    def pmaj(v):
        # [D] vector -> [P, NDC] partition-major (device column tile layout)
        return np.ascontiguousarray(np.asarray(v, f32).reshape(NDC, P).T)

    shared = {
        "wq": np.clip(np.asarray(inputs["Wq"], f32) * (scale * S_QK),
                      -240.0, 240.0).astype(FP8_NP),
        "wk": _q8(inputs["Wk"]),
        "wv": np.asarray(inputs["Wv"], f32).astype(BF16_NP),
        # weight layouts are partition-major so every DMA partition row is
        # one contiguous DRAM run (8x fewer descriptors than chunk-major)
        "wo": np.ascontiguousarray(
            np.asarray(inputs["Wo"], f32).astype(BF16_NP)
            .reshape(NDC, P, NOC, P).transpose(2, 1, 0, 3).reshape(NOC, P, D)),
        "bq": pmaj(np.asarray(inputs["bq"], f32) * scale),
        "bk": pmaj(inputs["bk"]),
        "bv": np.asarray(inputs["bv"], f32),
        "bo": pmaj(inputs["bo"]),
        "gate_wbf": np.ascontiguousarray(
            np.asarray(inputs["gate_w"], f32).astype(BF16_NP)
            .reshape(NDC, P, E).transpose(1, 0, 2)),
        "gate_wr": np.ascontiguousarray(
            (np.asarray(inputs["gate_w"], f32)
             - np.asarray(inputs["gate_w"], f32).astype(BF16_NP).astype(f32))
            .astype(BF16_NP).reshape(NDC, P, E).transpose(1, 0, 2)),
        "gate_b": np.asarray(inputs["gate_b"], f32),
        "ew1": _swi(np.ascontiguousarray(
            _q8(inputs["ew1"]).reshape(E, NDC, P, NFC, P)
            .transpose(0, 3, 2, 1, 4).reshape(E, NFC, P, D)), NDC // 2),
        "eb1": np.ascontiguousarray(
            np.asarray(inputs["eb1"], f32).reshape(E, NFC, P).transpose(2, 0, 1)),
        "ew2": _swi(np.ascontiguousarray(
            _q8(inputs["ew2"]).reshape(E, NFC, P, NOC, P)
            .transpose(0, 3, 2, 1, 4).reshape(E, NOC, P, FF)), NFC // 2),
        "eb2": np.ascontiguousarray(
            (np.asarray(inputs["eb2"], f32) * S_W)
            .reshape(E, NOC, P).transpose(2, 0, 1)),
        "ln1g": pmaj(inputs["ln1_g"]),
        "ln1b": pmaj(inputs["ln1_b"]),
        "ln2g": pmaj(inputs["ln2_g"]),
        "ln2b": pmaj(inputs["ln2_b"]),
    }

    in_maps = []
    for c in range(N_CORES):
        b, hh = c // 2, c % 2
        sl = slice(hh * TOK, (hh + 1) * TOK)
        # key/value tokens permuted so this core's own 512 tokens come first
        # (attention sums over j in any order; bj rows match the permutation)
        order = np.concatenate([np.arange(hh * TOK, (hh + 1) * TOK),
                                np.arange((1 - hh) * TOK, (2 - hh) * TOK)])
        srcT = np.ascontiguousarray(src[b].T)  # [D, T] f32
        fj = frac[b][order]   # [T] permuted
        m = dict(shared)
        src_perm = np.ascontiguousarray(srcT[:, order])
        m["srcT_full"] = src_perm.astype(BF16_NP)
        m["srcT8_full"] = np.clip(src_perm, -240.0, 240.0).astype(FP8_NP)
        m["res_own"] = np.ascontiguousarray(srcT[:, sl])
        m["bj"] = np.ascontiguousarray(
            (-(sum_b * scale) / fj).astype(f32).reshape(NJC, P).T)
        in_maps.append(m)
    return in_maps


def run_cores(in_maps, trace=False, **kwargs):
    nc = _build()
    return run_bass_kernel_spmd(nc, in_maps, core_ids=list(range(N_CORES)),
                                trace=trace, **kwargs)


def assemble_output(results):
    out = np.empty((B, T, D), np.float32)
    for c in range(N_CORES):
        b, hh = c // 2, c % 2
        out[b, hh * TOK:(hh + 1) * TOK] = results[c]["out"].T.astype(np.float32)
    return out


def kernel(**inputs):
    in_maps = prep_in_maps(inputs)
    res = run_cores(in_maps)
    return assemble_output(res.results)


if __name__ == "__main__":
    _build()
    print("build ok")


# revision 85
# speedup vs baseline: 1.0277x; 1.0277x over previous
"""Trainium2 Bass kernel for CustomTransformerEncoderMoELayer.

Sharding: pure data-parallel over (batch, token-half) -> 8 cores, no
collectives.  Core c handles batch c//2, tokens [512*(c%2), 512*(c%2+1)).
Each core runs an identical program on different data:

  - Q/K/V projections in feature-major layout (weights stationary),
    K/V computed for the full batch (needed for attention), Q for own tokens.
    Key/value tokens are host-permuted so the core's own tokens come first.
  - Attention: the frac factor (fj-fi)/(fi*fj+eps) equals 1/fi - 1/fj up to
    O(eps) (logit error <= 2e-4); the 1/fi term is constant per query and
    cancels in softmax, so only b_j = -(sum_b*scale)/fj remains, applied as
    a per-partition bias inside the ACT exp -- no Fs matrix, no multiplies.
    Softmax denominator obtained free via a ones-column appended to V.
  - Per-head softmax sums kept as a [16, T_own] tile (one head per
    partition): Ln/Exp run 16-lane, and the 1/sum row-broadcasts for all
    16 heads take 8 K=16 indicator matmuls instead of 16 K=1 ones.
  - LayerNorm in feature-major via ones-vector PE reductions and PE
    row-broadcasts; stats interleaved with the producing matmuls (LN1 with
    the out-projection, LN2 with the last expert) to keep the PE dense.
  - Gate in fp32 feature-major (lhsT=gate_w, one [4,512] psum, matmuls
    interleaved with LN1's affine chunks), transposed to token-major on
    the PE for the top-2 selection chain; combine weights broadcast
    through a DRAM bounce on the otherwise-idle gpsimd queue.
  - Dense MoE: all 4 experts computed for all tokens, combined with the
    (zero-masked) gate weights.  Both expert matmuls run in fp8(e4m3)
    DoubleRow perf mode (256-deep contraction per pass); weights are
    host-prescaled by 64 to stay in e4m3's normal range, with 1/64 folded
    into the ReLU activation scale (mm1) and the combine weights (mm2).
  - Output is returned feature-major [D, T_own]; the host transposes.
"""

import sys

sys.path.insert(0, "/opt/trn_rl_repo")

from contextlib import ExitStack

import ml_dtypes
import numpy as np

import concourse.bass as bass
import concourse.tile as tile
from concourse import bacc, mybir
from concourse.bass_utils import run_bass_kernel_spmd
from concourse.masks import make_identity

AF = mybir.ActivationFunctionType
ALU = mybir.AluOpType
F32 = mybir.dt.float32
BF16 = mybir.dt.bfloat16
F8 = mybir.dt.float8e4
DR = mybir.MatmulPerfMode.DoubleRow
# SwInterleave: same math as DoubleRow but weights pre-interleaved per column
# pair and column-reversed (the hardware's native LDW streaming order)
USE_SWI = True
DRW = mybir.MatmulPerfMode.DoubleRowSwInterleave if USE_SWI else DR
BF16_NP = ml_dtypes.bfloat16
FP8_NP = ml_dtypes.float8_e4m3

B, T, D = 4, 1024, 1024
H, HD, FF, E = 16, 64, 4096, 4
P = 128
TOK = 512  # tokens per core
NDC = D // P  # 8 feature chunks
NJC = T // P  # 8 key-token chunks
NFC = FF // P  # 32 FF chunks
NOC = D // P  # 8 output feature chunks
NTC = TOK // P  # 4 own-token chunks
N_CORES = 8
EPS_ATTN, EPS_LN = 1e-8, 1e-5
S_W = 64.0  # fp8 weight prescale (moves w into e4m3 normal range)
S_QK = 512.0  # prescale for wq (already carries the hd^-0.5 attention scale)


def _declare_io(nc):
    d = {}

    def din(name, shape, dtype):
        d[name] = nc.dram_tensor(name, shape, dtype, kind="ExternalInput").ap()

    din("srcT_full", [D, T], BF16)
    din("srcT8_full", [D, T], F8)
    din("res_own", [D, TOK], F32)
    din("wq", [D, D], F8)
    din("wk", [D, D], F8)
    din("wv", [D, D], BF16)
    din("wo", [NOC, P, D], BF16)
    # [D]-vector params arrive host-prearranged [P, NDC] (one contiguous
    # run per partition: 128 DMA descriptors instead of 1024 4-byte ones)
    din("bq", [P, NDC], F32)
    din("bk", [P, NDC], F32)
    din("bv", [D], F32)
    din("bo", [P, NDC], F32)
    din("gate_wbf", [P, NDC, E], BF16)
    din("gate_wr", [P, NDC, E], BF16)
    din("gate_b", [E], F32)
    din("ew1", [E, NFC, P, D], F8)
    din("eb1", [P, E, NFC], F32)
    din("ew2", [E, NOC, P, FF], F8)
    din("eb2", [P, E, NOC], F32)
    din("ln1g", [P, NDC], F32)
    din("ln1b", [P, NDC], F32)
    din("ln2g", [P, NDC], F32)
    din("ln2b", [P, NDC], F32)
    din("bj", [P, NJC], F32)
    d["out"] = nc.dram_tensor("out", [D, TOK], BF16, kind="ExternalOutput").ap()
    return d


def _bcast_ap(base, parts, free_len):
    """AP reading `free_len` contiguous elements at base, replicated on
    `parts` partitions (partition step 0)."""
    return bass.AP(tensor=base.tensor, offset=base.offset, ap=[[0, parts], [1, free_len]])


def _fm_layernorm(tc, nc, x_in, g_sb, b_sb, out_f32, out_q8, cst,
                  sq_pool, row_sb, bc_sb, producer=None, after_affine=None):
    """LayerNorm over the feature (partition x chunk) axis, feature-major.

    x_in(dc) -> [P, TOK] f32 view of chunk dc.  producer(dc), if given, emits
    the instructions that produce x_in(dc) (stats matmuls interleave with it).
    Stats run on bf16 casts (PE ones-reduction at full rate; the averaging
    washes out the rounding).  after_affine(dc) runs after each output chunk.
    """
    with tc.tile_pool(name="ln_row_ps", bufs=2, space="PSUM") as row_ps, \
         tc.tile_pool(name="ln_bc_ps", bufs=1, space="PSUM") as bc_ps:
        sum_ps = row_ps.tile([1, TOK], F32, name="lnrow", tag="lnrow")
        sumsq_ps = row_ps.tile([1, TOK], F32, name="lnrow", tag="lnrow")
        for dc in range(NDC):
            if producer is not None:
                producer(dc)
            xb = sq_pool.tile([P, TOK], BF16, name="xb", tag="xb")
            nc.vector.tensor_copy(xb, x_in(dc))
            nc.tensor.matmul(sum_ps, lhsT=cst["ones_col_bf"], rhs=xb,
                             start=(dc == 0), stop=(dc == NDC - 1))
            sqb = sq_pool.tile([P, TOK], BF16, name="sqb", tag="sqb")
            nc.vector.tensor_mul(sqb, xb, xb)
            nc.tensor.matmul(sumsq_ps, lhsT=cst["ones_col_bf"], rhs=sqb,
                             start=(dc == 0), stop=(dc == NDC - 1))
        stats2 = row_sb.tile([1, 2, TOK], F32, name="stats2", tag="stats2")
        mu_row = stats2[:, 0, :]
        nc.scalar.mul(mu_row, sum_ps, 1.0 / D)
        musq = row_sb.tile([1, TOK], F32, name="musq", tag="musq")
        nc.vector.tensor_mul(musq, mu_row, mu_row)
        var_row = row_sb.tile([1, TOK], F32, name="var_row", tag="var_row")
        nc.vector.scalar_tensor_tensor(out=var_row, in0=sumsq_ps, scalar=1.0 / D,
                                       in1=musq, op0=ALU.mult, op1=ALU.subtract)
        lnv_row = row_sb.tile([1, TOK], F32, name="lnv_row", tag="lnv_row")
        nc.scalar.activation(lnv_row, var_row, AF.Ln, bias=cst["eps_row"])
        # rstd = (var+eps)^-0.5 via exp/ln: stays in the natural_log_exp ACT
        # table set (no table switch around the attention/gate exps) and
        # avoids the low-precision Sqrt table
        nc.scalar.activation(stats2[:, 1, :], lnv_row, AF.Exp, scale=-0.5)

        # broadcast both rows (mu | rstd); one copy moves both to SBUF
        st_bc_ps = bc_ps.tile([P, 2, TOK], F32, name="lnbc", tag="lnbc")
        nc.tensor.matmul(st_bc_ps[:, 0, :], lhsT=cst["ones_row"], rhs=stats2[:, 0, :],
                         start=True, stop=True)
        nc.tensor.matmul(st_bc_ps[:, 1, :], lhsT=cst["ones_row"], rhs=stats2[:, 1, :],
                         start=True, stop=True)
        st_bc = bc_sb.tile([P, 2, TOK], F32, name="st_bc", tag="st_bc")
        nc.scalar.copy(st_bc, st_bc_ps)
        mu_bc = st_bc[:, 0, :]
        rstd_bc = st_bc[:, 1, :]

        for dc in range(NDC):
            t1 = sq_pool.tile([P, TOK], F32, name="sq", tag="sq")
            nc.vector.tensor_sub(t1, x_in(dc), mu_bc)
            t2 = sq_pool.tile([P, TOK], F32, name="sq", tag="sq")
            nc.vector.tensor_mul(t2, t1, rstd_bc)
            nc.scalar.activation(out_f32(dc), t2, AF.Identity,
                                 bias=b_sb[:, dc:dc + 1], scale=g_sb[:, dc:dc + 1])
            if out_q8 is not None:
                nc.vector.tensor_copy(out_q8(dc), out_f32(dc))
            if after_affine is not None:
                after_affine(dc)


def _emit_kernel(tc, nc, io):
    stk = ExitStack()
    with stk:
        # ---------------- constants / params (live whole kernel) ----------
        cpool = stk.enter_context(tc.tile_pool(name="const", bufs=1))
        cst = {}
        cst["ones_col_bf"] = cpool.tile([P, 1], BF16, name="ones_col_bf", tag="ones_col_bf")
        nc.vector.memset(cst["ones_col_bf"], 1.0)
        cst["ones_row"] = cpool.tile([1, P], F32, name="ones_row", tag="ones_row")
        nc.vector.memset(cst["ones_row"], 1.0)
        ident = cpool.tile([P, P], F32, name="ident", tag="ident")
        make_identity(nc, ident)
        cst["eps_row"] = cpool.tile([1, 1], F32, name="eps_row", tag="eps_row")
        nc.vector.memset(cst["eps_row"], EPS_LN)

        def col_tile(name, cols=NDC):
            return cpool.tile([P, cols], F32, name=name, tag=name)

        bq_sb = col_tile("bq")
        bk_sb = col_tile("bk")
        bo_sb = col_tile("bo")
        ln1g_sb = col_tile("ln1g")
        ln1b_sb = col_tile("ln1b")
        ln2g_sb = col_tile("ln2g")
        ln2b_sb = col_tile("ln2b")
        bj_sb = cpool.tile([P, NJC], F32, name="bj", tag="bj")
        eb1_sb = cpool.tile([P, E, NFC], F32, name="eb1", tag="eb1")
        eb2_sb = cpool.tile([P, E, NOC], F32, name="eb2", tag="eb2")
        gate_wbf_sb = cpool.tile([P, NDC, E], BF16, name="gate_wbf", tag="gate_wbf")
        gate_wr_sb = cpool.tile([P, NDC, E], BF16, name="gate_wr", tag="gate_wr")
        gate_b_bc = cpool.tile([P, E], F32, name="gate_b", tag="gate_b")
        bv_bc = cpool.tile([P, D], BF16, name="bv_bc", tag="bv_bc")

        def emit_const_loads():
            # all consts arrive host-prearranged partition-major: each DMA is
            # 128 contiguous runs, so the whole block is ~1K descriptors
            for t, name in ((bq_sb, "bq"), (bk_sb, "bk"), (bo_sb, "bo"),
                            (ln1g_sb, "ln1g"), (ln1b_sb, "ln1b"),
                            (ln2g_sb, "ln2g"), (ln2b_sb, "ln2b"),
                            (bj_sb, "bj"), (eb1_sb, "eb1"), (eb2_sb, "eb2"),
                            (gate_wbf_sb, "gate_wbf"), (gate_wr_sb, "gate_wr")):
                nc.sync.dma_start(out=t, in_=io[name])
            nc.sync.dma_start(out=gate_b_bc, in_=_bcast_ap(io["gate_b"], P, E))
            nc.gpsimd.dma_start(out=bv_bc, in_=_bcast_ap(io["bv"], P, D))

        # ---------------- persistent activations --------------------------
        per = stk.enter_context(tc.tile_pool(name="persist", bufs=1))
        xres = per.tile([P, NDC, TOK], F32, name="xres", tag="xres")
        xln = per.tile([P, NDC, TOK], F32, name="xln", tag="xln")
        xq8 = per.tile([P, NDC, TOK], F8, name="xq8", tag="xq8")
        ff = per.tile([P, NOC, TOK], F32, name="ff", tag="ff")

        sq_pool = stk.enter_context(tc.tile_pool(name="sq", bufs=3))
        row_sb = stk.enter_context(tc.tile_pool(name="row_sb", bufs=1))
        bc_sb = stk.enter_context(tc.tile_pool(name="bc_sb", bufs=1))
        # ================== attention ======================================
        with ExitStack() as astk:
            apool = astk.enter_context(tc.tile_pool(name="attn_sb", bufs=1))
            # Q zero-padded per head: even heads in rows 0:64 (zeros above),
            # odd heads in rows 64:128 (zeros below).  QK then contracts over
            # all 128 rows with the pair's shared K tile: the zeros kill the
            # other head's contribution, and the full-K matmul keeps the PE
            # activity monitor warm (K=64 streams throttle to half clock).
            QTp = apool.tile([P, H, TOK], BF16, name="QTp", tag="QTp")
            KT = apool.tile([P, NDC, T], BF16, name="KT", tag="KT")
            Vp = apool.tile([P, NJC, H, HD + 1], BF16, name="Vp", tag="Vp")
            attnT = apool.tile([P, NDC, TOK], BF16, name="attnT", tag="attnT")
            for jc in range(NJC):
                nc.vector.memset(Vp[:, jc, :, HD:HD + 1], 1.0)

            # ---- projections ----
            with ExitStack() as pstk:
                ppool = pstk.enter_context(tc.tile_pool(name="proj_sb", bufs=1))
                wpool = pstk.enter_context(tc.tile_pool(name="w_sb", bufs=2))
                mm_ps = pstk.enter_context(tc.tile_pool(name="proj_mm", bufs=3, space="PSUM"))
                srcT = ppool.tile([P, NDC, T], BF16, name="srcT", tag="srcT")
                srcT8 = ppool.tile([P, NDC, T], F8, name="srcT8", tag="srcT8")
                src_rearr = io["srcT_full"].rearrange("(c p) t -> p c t", p=P)
                src8_rearr = io["srcT8_full"].rearrange("(c p) t -> p c t", p=P)

                def load_w(name, dtype):
                    w = wpool.tile([P, NDC, D], dtype, tag="w", name="w")
                    w_rearr = io[name].rearrange("(c p) o -> p c o", p=P)
                    for dc in range(NDC):
                        nc.sync.dma_start(out=w[:, dc:dc + 1, :], in_=w_rearr[:, dc:dc + 1, :])
                    return w

                # one engine queue per input stream so K (wk8+srcT8), Q (wq8)
                # and V (srcT bf16 + wv) are never serialized behind each
                # other: sync carries the weights, scalar srcT8, gpsimd srcT
                wk = wpool.tile([P, NDC, D], F8, tag="w", name="w")
                wk_rearr = io["wk"].rearrange("(c p) o -> p c o", p=P)
                for dc in range(NDC):
                    nc.sync.dma_start(out=wk[:, dc:dc + 1, :], in_=wk_rearr[:, dc:dc + 1, :])
                    nc.scalar.dma_start(out=srcT8[:, dc:dc + 1, :], in_=src8_rearr[:, dc:dc + 1, :])
                    nc.gpsimd.dma_start(out=srcT[:, dc:dc + 1, :], in_=src_rearr[:, dc:dc + 1, :])
                emit_const_loads()

                # K projection: feature-major, full batch, fp8 DoubleRow
                # (wk host-prescaled by S_W; descale rides the ACT bias pass)
                for oc in range(NDC):
                    for th in range(T // TOK):
                        ps = mm_ps.tile([P, TOK], F32, name="mm", tag="mm")
                        for d2 in range(NDC // 2):
                            nc.tensor.matmul(ps, lhsT=wk[:, 2 * d2:2 * d2 + 2, oc * P:(oc + 1) * P],
                                             rhs=srcT8[:, 2 * d2:2 * d2 + 2, th * TOK:(th + 1) * TOK],
                                             start=(d2 == 0), stop=(d2 == NDC // 2 - 1),
                                             perf_mode=DR)
                        nc.scalar.activation(KT[:, oc, th * TOK:(th + 1) * TOK], ps,
                                             AF.Identity, bias=bk_sb[:, oc:oc + 1],
                                             scale=1.0 / S_W)
                # Q projection (own tokens = first TOK of the permuted order;
                # wq/bq pre-scaled by hd^-0.5 on host, wq by S_QK for fp8)
                for h in range(H):
                    z0 = (h % 2) * HD  # zeros live in the OTHER half
                    nc.vector.memset(QTp[HD - z0:P - z0, h, :], 0.0)
                wq = load_w("wq", F8)
                for oc in range(NDC):
                    ps = mm_ps.tile([P, TOK], F32, name="mm", tag="mm")
                    for d2 in range(NDC // 2):
                        nc.tensor.matmul(ps, lhsT=wq[:, 2 * d2:2 * d2 + 2, oc * P:(oc + 1) * P],
                                         rhs=srcT8[:, 2 * d2:2 * d2 + 2, 0:TOK],
                                         start=(d2 == 0), stop=(d2 == NDC // 2 - 1),
                                         perf_mode=DR)
                    nc.scalar.activation(QTp[0:HD, 2 * oc, :], ps[0:HD, :],
                                         AF.Identity, bias=bq_sb[0:HD, oc:oc + 1],
                                         scale=1.0 / S_QK)
                    nc.scalar.activation(QTp[HD:P, 2 * oc + 1, :], ps[HD:P, :],
                                         AF.Identity, bias=bq_sb[HD:P, oc:oc + 1],
                                         scale=1.0 / S_QK)
                # V projection: token-major (src chunk stationary), full batch
                wv = load_w("wv", BF16)
                for jc in range(NJC):
                    for nh in range(D // TOK):
                        ps = mm_ps.tile([P, TOK], F32, name="mm", tag="mm")
                        for dc in range(NDC):
                            nc.tensor.matmul(ps, lhsT=srcT[:, dc, jc * P:(jc + 1) * P],
                                             rhs=wv[:, dc, nh * TOK:(nh + 1) * TOK],
                                             start=(dc == 0), stop=(dc == NDC - 1))
                        nc.vector.tensor_add(
                            Vp[:, jc, nh * 8:(nh + 1) * 8, 0:HD],
                            ps.rearrange("p (a b) -> p a b", a=8),
                            bv_bc[:, nh * TOK:(nh + 1) * TOK].rearrange("p (a b) -> p a b", a=8))

            # prefetch the residual while QK/PV runs (persist tile: no extra SBUF)
            res_rearr = io["res_own"].rearrange("(c p) t -> p c t", p=P)
            for dc in range(NDC):
                nc.sync.dma_start(out=xres[:, dc:dc + 1, :], in_=res_rearr[:, dc:dc + 1, :])



            # ---- attention core: head pairs packed via tile_position ----
            # probs ∝ exp(K^T Q * scale + b_j): the frac factor's query term
            # cancels in softmax and the key term b_j rides the ACT exp as a
            # per-partition bias -- no Fs matrix multiply at all.
            sums_pool = astk.enter_context(tc.tile_pool(name="sums_sb", bufs=1))
            sums_all = sums_pool.tile([1, H, TOK], F32, name="sums_all", tag="sums_all")
            with ExitStack() as astk2:
                e0_pool = astk2.enter_context(tc.tile_pool(name="e0_sb", bufs=6))
                s_ps_pool = astk2.enter_context(tc.tile_pool(name="s_ps", bufs=2, space="PSUM"))
                att_ps_pool = astk2.enter_context(tc.tile_pool(name="att_ps", bufs=3, space="PSUM"))
                bc_ps_pool = astk2.enter_context(tc.tile_pool(name="bc_ps", bufs=1, space="PSUM"))
                rcp_pool = astk2.enter_context(tc.tile_pool(name="rcp_sb", bufs=2))

                def norm_chunk(dch):
                    # pair dch's heads fill attnT chunk dch: broadcast the two
                    # raw sums, one 128-lane reciprocal + multiply -- rides
                    # inside the ring so out-proj starts right after pair 7
                    ha, hb = 2 * dch, 2 * dch + 1
                    bc_ps = bc_ps_pool.tile([P, TOK], F32, name="bc", tag="bc")
                    nc.tensor.matmul(bc_ps[0:HD, :], lhsT=cst["ones_row"][:, 0:HD],
                                     rhs=sums_all[0:1, ha, :], start=True, stop=True)
                    nc.tensor.matmul(bc_ps[HD:P, :], lhsT=cst["ones_row"][:, 0:HD],
                                     rhs=sums_all[0:1, hb, :], start=True, stop=True)
                    rcp = rcp_pool.tile([P, TOK], F32, name="rcp", tag="rcp")
                    nc.vector.reciprocal(rcp, bc_ps)
                    nc.vector.tensor_tensor(attnT[:, dch, :], attnT[:, dch, :], rcp,
                                            op=ALU.mult)

                for hp2 in range(H // 2):
                    ha, hb = 2 * hp2, 2 * hp2 + 1
                    att_a = att_ps_pool.tile([HD + 1, TOK], F32, name="att", tag="att")
                    att_b = att_ps_pool.tile([HD + 1, TOK], F32, name="att", tag="att")
                    exp_tiles = []

                    def emit_pv(jc, att_a=att_a, att_b=att_b, exp_tiles=exp_tiles,
                                ha=ha, hb=hb):
                        et = exp_tiles[jc]
                        nc.tensor.matmul(att_a, lhsT=Vp[:, jc, ha, :], rhs=et[:, 0, :],
                                         start=(jc == 0), stop=(jc == NJC - 1))
                        nc.tensor.matmul(att_b, lhsT=Vp[:, jc, hb, :], rhs=et[:, 1, :],
                                         start=(jc == 0), stop=(jc == NJC - 1))

                    for jc in range(NJC):
                        s_ps = s_ps_pool.tile([P, 2, TOK], F32, name="s", tag="s")
                        nc.tensor.matmul(s_ps[:, 0, :], lhsT=KT[:, hp2, jc * P:(jc + 1) * P],
                                         rhs=QTp[:, ha, :], start=True, stop=True)
                        nc.tensor.matmul(s_ps[:, 1, :], lhsT=KT[:, hp2, jc * P:(jc + 1) * P],
                                         rhs=QTp[:, hb, :], start=True, stop=True)
                        et = e0_pool.tile([P, 2, TOK], BF16, name="e0", tag="e0")
                        nc.scalar.activation(et, s_ps, AF.Exp, bias=bj_sb[:, jc:jc + 1])
                        exp_tiles.append(et)
                        if jc >= 2:
                            emit_pv(jc - 2)
                    emit_pv(NJC - 2)
                    emit_pv(NJC - 1)
                    # stage unnormalized head outputs + softmax sums; all
                    # normalization is batched after the loop (one Ln + one
                    # Exp for all 16 heads -- per-pair Ln/Exp thrashed the
                    # ACT table sets, ~3 TABLE_LOADs per pair)
                    for i, (att, h) in enumerate(((att_a, ha), (att_b, hb))):
                        nc.vector.tensor_copy(attnT[i * HD:(i + 1) * HD, hp2, :], att[0:HD, :])
                        nc.vector.tensor_copy(sums_all[0:1, h, :], att[HD:HD + 1, :])
                    norm_chunk(hp2)

            # gate psum lives from LN1's affine chunks to the gate section
            gate_ps_pool = stk.enter_context(tc.tile_pool(name="gate_ps", bufs=1, space="PSUM"))
            g_ps = gate_ps_pool.tile([E, TOK], F32, name="g_ps", tag="g_ps")

            # ---- output projection + residual + LN1 (stats interleaved) ----
            with ExitStack() as ostk:
                mm_ps = ostk.enter_context(tc.tile_pool(name="out_mm", bufs=2, space="PSUM"))
                wo_pool = ostk.enter_context(tc.tile_pool(name="wo_sb", bufs=3))
                # wo streams as per-oc column tiles: the first psum group
                # needs 256KB, not the whole 2MB tensor
                wo_tiles = []
                for oc in range(NOC):
                    woc = wo_pool.tile([P, NDC, P], BF16, name="woc", tag="woc")
                    nc.sync.dma_start(out=woc, in_=io["wo"][oc].rearrange("p (c n) -> p c n", c=NDC))
                    wo_tiles.append(woc)

                def ln1_producer(oc):
                    ps = mm_ps.tile([P, TOK], F32, name="mm", tag="mm")
                    for dc in range(NDC):
                        nc.tensor.matmul(ps, lhsT=wo_tiles[oc][:, dc, :],
                                         rhs=attnT[:, dc, :],
                                         start=(dc == 0), stop=(dc == NDC - 1))
                    nc.vector.scalar_tensor_tensor(out=xres[:, oc, :], in0=ps,
                                                   scalar=bo_sb[:, oc:oc + 1],
                                                   in1=xres[:, oc, :],
                                                   op0=ALU.add, op1=ALU.add)

                _fm_layernorm(tc, nc, lambda dc: xres[:, dc, :], ln1g_sb, ln1b_sb,
                              lambda dc: xln[:, dc, :], lambda dc: xq8[:, dc, :],
                              cst, sq_pool, row_sb, bc_sb, producer=ln1_producer)

        # combine weights (row-broadcast), needed from gate through MoE
        cbc_pool = stk.enter_context(tc.tile_pool(name="cbc_pool", bufs=1))
        cbc = cbc_pool.tile([P, E, TOK], F32, name="cbc", tag="cbc")

        # bf16 value + residual of xln for the gate's split-bf16 logits
        # (xbf@gbf + xbf@gr + xr@gbf; products are exact in the f32 psum, so
        # only the ~2^-18 xr@gr term is dropped)
        xsplit = stk.enter_context(tc.tile_pool(name="xsplit", bufs=1))
        xbf = xsplit.tile([P, NDC, TOK], BF16, name="xbf", tag="xbf")
        xr = xsplit.tile([P, NDC, TOK], BF16, name="xr", tag="xr")
        for dc in range(NDC):
            nc.vector.tensor_copy(xbf[:, dc, :], xln[:, dc, :])
            nc.vector.tensor_sub(xr[:, dc, :], xln[:, dc, :], xbf[:, dc, :])

        # ================== MoE (dense, all experts) + LN2 + output ========
        with ExitStack() as mstk:
            h_pool = mstk.enter_context(tc.tile_pool(name="hT", bufs=2))
            w1_pool = mstk.enter_context(tc.tile_pool(name="ew1_sb", bufs=10))
            w2_pool = mstk.enter_context(tc.tile_pool(name="ew2_sb", bufs=3))
            ytmp_pool = mstk.enter_context(tc.tile_pool(name="ytmp", bufs=2))
            mm_ps = mstk.enter_context(tc.tile_pool(name="moe_mm", bufs=2, space="PSUM"))

            def expert_w1_load(e):
                tiles = []
                for fc in range(NFC):
                    w1 = w1_pool.tile([P, NDC, P], F8, name="w1", tag="w1")
                    nc.sync.dma_start(out=w1, in_=io["ew1"][e, fc].rearrange("p (c n) -> p c n", c=NDC))
                    tiles.append(w1)
                return tiles

            def expert_h(e, w1_tiles, extra=None):
                h_all = h_pool.tile([P, NFC, TOK], F8, name="h_all", tag="h_all")
                for fc in range(NFC):
                    w1 = w1_tiles[fc]
                    h_ps = mm_ps.tile([P, TOK], F32, name="mm", tag="mm")
                    for d2 in range(NDC // 2):
                        nc.tensor.matmul(h_ps, lhsT=w1[:, 2 * d2:2 * d2 + 2, :],
                                         rhs=xq8[:, 2 * d2:2 * d2 + 2, :],
                                         start=(d2 == 0), stop=(d2 == NDC // 2 - 1),
                                         perf_mode=DRW)
                    nc.scalar.activation(h_all[:, fc, :], h_ps, AF.Relu,
                                         bias=eb1_sb[:, e, fc:fc + 1], scale=1.0 / S_W)
                    if extra is not None:
                        extra(fc)
                return h_all

            def w2_load(e, oc):
                w2 = w2_pool.tile([P, NFC, P], F8, name="w2", tag="w2")
                nc.gpsimd.dma_start(out=w2, in_=io["ew2"][e, oc].rearrange("p (c n) -> p c n", c=NFC))
                return w2

            def expert_y(e, h_all, oc, w2=None):
                if w2 is None:
                    w2 = w2_load(e, oc)
                y_ps = mm_ps.tile([P, TOK], F32, name="mm", tag="mm")
                for f2 in range(NFC // 2):
                    nc.tensor.matmul(y_ps, lhsT=w2[:, 2 * f2:2 * f2 + 2, :],
                                     rhs=h_all[:, 2 * f2:2 * f2 + 2, :],
                                     start=(f2 == 0), stop=(f2 == NFC // 2 - 1),
                                     perf_mode=DRW)
                if e == 0:
                    nc.vector.scalar_tensor_tensor(out=ff[:, oc, :], in0=y_ps,
                                                   scalar=eb2_sb[:, e, oc:oc + 1],
                                                   in1=cbc[:, e, :], op0=ALU.add, op1=ALU.mult)
                else:
                    yt = ytmp_pool.tile([P, TOK], F32, name="yt", tag="yt")
                    nc.vector.scalar_tensor_tensor(out=yt, in0=y_ps,
                                                   scalar=eb2_sb[:, e, oc:oc + 1],
                                                   in1=cbc[:, e, :], op0=ALU.add, op1=ALU.mult)
                    nc.vector.tensor_add(ff[:, oc, :], ff[:, oc, :], yt)

            # prefetch expert-0 weights before the gate's DVE chain queues up
            w1_first = expert_w1_load(0)

            def gate_mm(fc):
                # split-bf16 gate logit matmuls ride inside expert-0's h stream
                if fc < NDC:
                    for i, (w, x) in enumerate(((gate_wbf_sb, xbf), (gate_wr_sb, xbf),
                                                (gate_wbf_sb, xr))):
                        nc.tensor.matmul(g_ps, lhsT=w[:, fc, :], rhs=x[:, fc, :],
                                         start=(fc == 0 and i == 0),
                                         stop=(fc == NDC - 1 and i == 2))

            h_all0 = expert_h(0, w1_first, extra=gate_mm)
            # expert-0's first two w2 tiles stream before the gate's gpsimd
            # DMAs (only 2 outstanding so the cbc bounce is never queued
            # behind a slot-blocked w2 load)
            w2_pre = [w2_load(0, 0), w2_load(0, 1)]

            # ================== gate + top-2 routing (fp32) ====================
            # logits accumulated during expert-0 h; transpose to token-major
            # on the PE for the selection chain.  DRAM bounce rides the
            # gpsimd queue so it is not stuck behind w1/w2 streams.
            with ExitStack() as gstk:
                gsb = gstk.enter_context(tc.tile_pool(name="gate_sb", bufs=3))
                gsmall = gstk.enter_context(tc.tile_pool(name="gate_small", bufs=2))
                tp4_ps_pool = gstk.enter_context(tc.tile_pool(name="tp4_ps", bufs=2, space="PSUM"))
                dram_pool = gstk.enter_context(tc.tile_pool(name="cdram", bufs=1, space="DRAM"))
                c_dram = dram_pool.tile([E, TOK], F32, name="c_dram", tag="c_dram")

                lgT = gsb.tile([E, TOK], F32, name="lgT", tag="lgT")
                nc.scalar.copy(lgT, g_ps)
                for tcn in range(NTC):
                    tp4 = tp4_ps_pool.tile([P, E], F32, name="tp4", tag="tp4")
                    nc.tensor.transpose(tp4, lgT[:, tcn * P:(tcn + 1) * P], ident[0:E, 0:E])
                    lg = gsb.tile([P, E], F32, name="lg", tag="lg")
                    nc.vector.tensor_add(lg, tp4, gate_b_bc)
                    m = gsmall.tile([P, 1], F32, name="m", tag="m")
                    nc.vector.reduce_max(m, lg, axis=mybir.AxisListType.X)
                    negm = gsmall.tile([P, 1], F32, name="negm", tag="negm")
                    nc.vector.tensor_scalar(negm, m, -1.0, None, op0=ALU.mult)
                    et = gsb.tile([P, E], F32, name="et", tag="et")
                    nc.scalar.activation(et, lg, AF.Exp, bias=negm)
                    ssum = gsmall.tile([P, 1], F32, name="ssum", tag="ssum")
                    nc.vector.reduce_sum(ssum, et, axis=mybir.AxisListType.X)
                    rinv = gsmall.tile([P, 1], F32, name="rinv", tag="rinv")
                    nc.vector.reciprocal(rinv, ssum)
                    pt = gsb.tile([P, E], F32, name="pt", tag="pt")
                    nc.vector.tensor_scalar(pt, et, rinv, None, op0=ALU.mult)
                    # pairwise is_ge: [ge01, ge12, ge23], [ge02, ge13], [ge03]
                    ge1 = gsb.tile([P, 3], F32, name="ge1", tag="ge1")
                    nc.vector.tensor_tensor(ge1, pt[:, 0:3], pt[:, 1:4], op=ALU.is_ge)
                    ge2 = gsb.tile([P, 2], F32, name="ge2", tag="ge2")
                    nc.vector.tensor_tensor(ge2, pt[:, 0:2], pt[:, 2:4], op=ALU.is_ge)
                    ge3 = gsb.tile([P, 1], F32, name="ge3", tag="ge3")
                    nc.vector.tensor_tensor(ge3, pt[:, 0:1], pt[:, 3:4], op=ALU.is_ge)
                    cnt = gsb.tile([P, E], F32, name="cnt", tag="cnt")
                    tmp = gsmall.tile([P, 1], F32, name="tmp", tag="tmp")
                    # cnt0 = 3 - ge01 - ge02 - ge03
                    nc.vector.tensor_add(tmp, ge1[:, 0:1], ge2[:, 0:1])
                    nc.vector.tensor_add(tmp, tmp, ge3[:, 0:1])
                    nc.vector.tensor_scalar(cnt[:, 0:1], tmp, -1.0, 3.0, op0=ALU.mult, op1=ALU.add)
                    # cnt1 = 2 + ge01 - ge12 - ge13
                    nc.vector.tensor_sub(tmp, ge1[:, 0:1], ge1[:, 1:2])
                    nc.vector.tensor_sub(tmp, tmp, ge2[:, 1:2])
                    nc.vector.tensor_scalar(cnt[:, 1:2], tmp, 2.0, None, op0=ALU.add)
                    # cnt2 = 1 + ge02 + ge12 - ge23
                    nc.vector.tensor_add(tmp, ge2[:, 0:1], ge1[:, 1:2])
                    nc.vector.tensor_sub(tmp, tmp, ge1[:, 2:3])
                    nc.vector.tensor_scalar(cnt[:, 2:3], tmp, 1.0, None, op0=ALU.add)
                    # cnt3 = ge03 + ge13 + ge23
                    nc.vector.tensor_add(tmp, ge3[:, 0:1], ge2[:, 1:2])
                    nc.vector.tensor_add(cnt[:, 3:4], tmp, ge1[:, 2:3])
                    mask = gsb.tile([P, E], F32, name="mask", tag="mask")
                    nc.vector.tensor_scalar(mask, cnt, 1.5, None, op0=ALU.is_le)
                    csb = gsb.tile([P, E], F32, name="csb", tag="csb")
                    # 1/S_W folded here: cbc carries combine/S_W so the
                    # host-prescaled fp8 mm2 psum needs no extra descale op
                    nc.vector.scalar_tensor_tensor(out=csb, in0=pt, scalar=1.0 / S_W,
                                                   in1=mask, op0=ALU.mult, op1=ALU.mult)
                    nc.gpsimd.dma_start(out=c_dram[:, tcn * P:(tcn + 1) * P].rearrange("e t -> t e"),
                                        in_=csb)
                for e in range(E):
                    nc.gpsimd.dma_start(out=cbc[:, e, :], in_=_bcast_ap(c_dram[e:e + 1, :], P, TOK))

            # experts pipeline: w1 for expert e+1 streams while expert e's
            # second matmul runs (w2 rides the vector queue, w1 the sync one)
            w1_next = expert_w1_load(1)
            for e in range(E - 1):
                h_all = h_all0 if e == 0 else expert_h(e, w1_next)
                if e > 0:
                    w1_next = expert_w1_load(e + 1)
                for oc in range(NOC):
                    expert_y(e, h_all, oc,
                             w2=w2_pre[oc] if (e == 0 and oc < len(w2_pre)) else None)
            # last expert: y-chunks + x2 = xln + ff feed LN2 stats directly
            h_last = expert_h(E - 1, w1_next)

            def ln2_producer(oc):
                expert_y(E - 1, h_last, oc)
                nc.vector.tensor_add(ff[:, oc, :], ff[:, oc, :], xln[:, oc, :])

            obf = h_pool.tile([P, NDC, TOK], BF16, name="obf", tag="obf", bufs=1)
            out_rearr = io["out"].rearrange("(c p) t -> p c t", p=P)

            def ln2_after(dc):
                # output leaves feature-major bf16; the host transposes/upcasts
                nc.sync.dma_start(out=out_rearr[:, dc:dc + 1, :], in_=obf[:, dc:dc + 1, :])

            _fm_layernorm(tc, nc, lambda dc: ff[:, dc, :], ln2g_sb, ln2b_sb,
                          lambda dc: obf[:, dc, :], None,
                          cst, sq_pool, row_sb, bc_sb,
                          producer=ln2_producer, after_affine=ln2_after)


_CACHE = {}


def _build():
    if "nc" in _CACHE:
        return _CACHE["nc"]
    nc = bacc.Bacc("TRN2", target_bir_lowering=False, debug=False, num_devices=N_CORES)
    io = _declare_io(nc)
    with tile.TileContext(nc) as tc:
        _emit_kernel(tc, nc, io)
    nc.compile()
    _CACHE["nc"] = nc
    return nc


def _q8(w):
    return np.clip(np.asarray(w, np.float32) * S_W, -240.0, 240.0).astype(FP8_NP)


def _swi(a, npair):
    """SwInterleave weight layout: per k-pair, columns interleaved
    (A,B alternating) and column-reversed -- flat[d2, 2i+j] = chunk_{2d2+j}
    column (127-i).  No-op passthrough shape-wise."""
    if not USE_SWI:
        return a
    shp = a.shape[:-1]
    v = a.reshape(*shp, npair, 2, P)[..., ::-1]
    v = np.swapaxes(v, -1, -2)
    return np.ascontiguousarray(v.reshape(*shp, npair * 2 * P))


def prep_in_maps(inputs):
    f32 = np.float32
    src = np.asarray(inputs["src"], f32)
    frac = np.asarray(inputs["frac"], f32)
    attn_bias = np.asarray(inputs["attn_bias"], f32)
    scale = f32(HD ** -0.5)
    sum_b = np.sum(attn_bias, dtype=f32)

    def pmaj(v):
        # [D] vector -> [P, NDC] partition-major (device column tile layout)
        return np.ascontiguousarray(np.asarray(v, f32).reshape(NDC, P).T)

    shared = {
        "wq": np.clip(np.asarray(inputs["Wq"], f32) * (scale * S_QK),
                      -240.0, 240.0).astype(FP8_NP),
        "wk": _q8(inputs["Wk"]),
        "wv": np.asarray(inputs["Wv"], f32).astype(BF16_NP),
        # weight layouts are partition-major so every DMA partition row is
        # one contiguous DRAM run (8x fewer descriptors than chunk-major)
        "wo": np.ascontiguousarray(
            np.asarray(inputs["Wo"], f32).astype(BF16_NP)
            .reshape(NDC, P, NOC, P).transpose(2, 1, 0, 3).reshape(NOC, P, D)),
        "bq": pmaj(np.asarray(inputs["bq"], f32) * scale),
        "bk": pmaj(inputs["bk"]),
        "bv": np.asarray(inputs["bv"], f32),
        "bo": pmaj(inputs["bo"]),
        "gate_wbf": np.ascontiguousarray(
            np.asarray(inputs["gate_w"], f32).astype(BF16_NP)
            .reshape(NDC, P, E).transpose(1, 0, 2)),
        "gate_wr": np.ascontiguousarray(
            (np.asarray(inputs["gate_w"], f32)
             - np.asarray(inputs["gate_w"], f32).astype(BF16_NP).astype(f32))
            .astype(BF16_NP).reshape(NDC, P, E).transpose(1, 0, 2)),
        "gate_b": np.asarray(inputs["gate_b"], f32),
        "ew1": _swi(np.ascontiguousarray(
            _q8(inputs["ew1"]).reshape(E, NDC, P, NFC, P)
            .transpose(0, 3, 2, 1, 4).reshape(E, NFC, P, D)), NDC // 2),
        "eb1": np.ascontiguousarray(
            np.asarray(inputs["eb1"], f32).reshape(E, NFC, P).transpose(2, 0, 1)),
        "ew2": _swi(np.ascontiguousarray(
            _q8(inputs["ew2"]).reshape(E, NFC, P, NOC, P)
            .transpose(0, 3, 2, 1, 4).reshape(E, NOC, P, FF)), NFC // 2),
        "eb2": np.ascontiguousarray(
            (np.asarray(inputs["eb2"], f32) * S_W)
            .reshape(E, NOC, P).transpose(2, 0, 1)),
        "ln1g": pmaj(inputs["ln1_g"]),
        "ln1b": pmaj(inputs["ln1_b"]),
        "ln2g": pmaj(inputs["ln2_g"]),
        "ln2b": pmaj(inputs["ln2_b"]),
    }

    in_maps = []
    for c in range(N_CORES):
        b, hh = c // 2, c % 2
        sl = slice(hh * TOK, (hh + 1) * TOK)
        # key/value tokens permuted so this core's own 512 tokens come first
        # (attention sums over j in any order; bj rows match the permutation)
        order = np.concatenate([np.arange(hh * TOK, (hh + 1) * TOK),
                                np.arange((1 - hh) * TOK, (2 - hh) * TOK)])
        srcT = np.ascontiguousarray(src[b].T)  # [D, T] f32
        fj = frac[b][order]   # [T] permuted
        m = dict(shared)
        src_perm = np.ascontiguousarray(srcT[:, order])
        m["srcT_full"] = src_perm.astype(BF16_NP)
        m["srcT8_full"] = np.clip(src_perm, -240.0, 240.0).astype(FP8_NP)
        m["res_own"] = np.ascontiguousarray(srcT[:, sl])
        m["bj"] = np.ascontiguousarray(
            (-(sum_b * scale) / fj).astype(f32).reshape(NJC, P).T)
        in_maps.append(m)
    return in_maps


def run_cores(in_maps, trace=False, **kwargs):
    nc = _build()
    return run_bass_kernel_spmd(nc, in_maps, core_ids=list(range(N_CORES)),
                                trace=trace, **kwargs)


def assemble_output(results):
    out = np.empty((B, T, D), np.float32)
    for c in range(N_CORES):
        b, hh = c // 2, c % 2
        out[b, hh * TOK:(hh + 1) * TOK] = results[c]["out"].T.astype(np.float32)
    return out


def kernel(**inputs):
    in_maps = prep_in_maps(inputs)
    res = run_cores(in_maps)
    return assemble_output(res.results)


if __name__ == "__main__":
    _build()
    print("build ok")
